# revision 29
# baseline (speedup 1.0000x reference)
"""Trainium2 Bass kernel for nn_MemoryCell (scatter_memory), v5.

Full-input contract: kernel(**inputs) takes the complete (unsharded) numpy
inputs and returns the full [NB*B, H] output.

Math (B == H == 1024, NB == 5, T == 128):
    enc  = features[:, 0, :]                         # [B, H] - only slice used
    h    = states.reshape(NB, H)
    gate = sigmoid(enc @ (h + keys).T)               # [B, NB]
    pre  = (h @ Uw.T + keys @ Vw.T)[:, None, :] + (enc @ Ww.T)[None, :, :]
    cand = where(pre >= 0, pre, prelu_a * pre)
    new[i, b, j] = h[i, j] + gate[j, i] * cand[i, b, j]   # B==H broadcast quirk
    out  = sign(new) with exact zeros -> +1, reshaped [NB*B, H]

Because gate > 0 and (for prelu slope a > 0) new is monotone in ew =
enc @ Ww.T, each output element is a pure threshold test:

    sign(new[i, b, j]) = +1  iff  ew[j, b] + nthr[j, i] >= 0
    nthr = huv + (h / s) * (1 + exp(-z)),  s = a if h > 0 else 1

nthr is a tiny [H, NB] tensor: the host computes it exactly (float64) from
the small operands.  The device does the big work: stream enc (fp16), run
the ew matmul, apply one threshold compare per (block, j-group, bank).

Sharding is 2D: 4 j-shards (256 features) x 2 b-halves (512 batch), so
each core streams only HALF of enc (1.05 MB) + its Ww j-shard (0.52 MB)
fp16: 1.58 MB in, 0.65 MB out per core (vs 36 MB naive / 7.2 MB for the
prior fp16-hi/lo kernel).

Hardware notes baked into the structure (from perfetto traces):
  * PSUM bank reads serialize across engines per instruction, so the ACT
    and DVE tail lanes own disjoint PSUM banks; widths 320/192 balance
    ACT (0.83 ns/elem + ~160ns/op) vs DVE (1.04 ns/elem + ~220ns/op).
  * ACT and DVE lanes write separate SBUF output tiles (a shared tile
    serializes the writers through the framework's WAW ordering).
  * enc ships as 5 rings (4 x 80 cols for the ACT banks + 192 for the
    DVE banks) and Ww as 4 k-split rings: concurrent rings keep the DMA
    descriptor pipelines fed (a single ring is capped at ~230 GB/s by
    per-descriptor overhead), small rings complete first under the DMA
    engines' round-robin (so first-needed data is in small rings, the
    last-needed DVE ring is the big one), and first-wave descriptor
    gens issue from 3 engines in parallel (each dma_start costs ~650ns
    serialized per descriptor-gen path).
  * PE warm-up transposes bridge the DMA wait; the PE clock needs ~6us
    of continuous activity to reach full speed, so early ew matmuls run
    at the mid p-state and the warm-up just keeps the ramp going.
Measured 109 sign flips vs the 524-flip (2e-2 rel err) budget.
"""

import numpy as np

H = 1024
NB = 5
B = 1024
NCORES = 8
NJ = 4                    # j shards
JS = H // NJ              # 256 features per core (2 PE groups of 128)
HB = B // 2               # 512 batch columns per core
KC = H // 128             # 8 contraction chunks
AW = 320                  # tail columns on ACT per group (rest on DVE)
DW = HB - AW              # 192
RW = [80, 80, 80, 80, 192]  # enc rings: 4 small (ACT bank) + 1 (DVE)
WARMUP = 20

_NC_CACHE = {}


def _build_nc():
    from concourse import bacc, mybir
    import concourse.tile as tile
    from concourse.masks import make_identity

    f32 = mybir.dt.float32
    f16 = mybir.dt.float16
    i8 = mybir.dt.int8
    AF = mybir.ActivationFunctionType
    ALU = mybir.AluOpType

    nc = bacc.Bacc("TRN2", debug=False, num_devices=NCORES)

    # wt ships as 4 k-split rings so the small rings complete first under
    # the DMA engines' round-robin (completion order ~ ring size)
    wt_d = [nc.dram_tensor(f"wt{p}", [128, 2, 2, 128], f16,
                           kind="ExternalInput").ap() for p in range(4)]
    thr_d = [nc.dram_tensor(f"thr{g}", [128, 16], f32,
                            kind="ExternalInput").ap() for g in range(2)]
    enc_d = [nc.dram_tensor(f"enc{q}", [128, KC, RW[q]], f16,
                            kind="ExternalInput").ap() for q in range(5)]
    oa_d = nc.dram_tensor("oa", [128, 2, NB, AW], i8, kind="ExternalOutput").ap()
    od_d = nc.dram_tensor("od", [128, 2, NB, DW], i8, kind="ExternalOutput").ap()

    with tile.TileContext(nc) as tc:
        with (
            tc.tile_pool(name="res", bufs=1) as res,
            tc.tile_pool(name="ps", bufs=1, space="PSUM") as ps,
        ):
            # identity for PE warm-up FIRST: make_identity runs on gpsimd,
            # which must not be stuck behind its DMA descriptor gens
            identity = res.tile([128, 128], f32, name="identity")
            make_identity(nc, identity)

            # ---- input DMAs; first wave issues from 3 engines in parallel
            wt = [res.tile([128, 2, 2, 128], f16, name=f"wt{p}")
                  for p in range(4)]
            thr = [res.tile([128, 16], f32, name=f"thr{g}")
                   for g in range(2)]
            enc = [res.tile([128, KC, RW[q]], f16, name=f"enc{q}")
                   for q in range(5)]
            nc.sync.dma_start(wt[0], wt_d[0])
            nc.scalar.dma_start(enc[0], enc_d[0])
            nc.scalar.dma_start(enc[1], enc_d[1])
            nc.gpsimd.dma_start(enc[2], enc_d[2])
            nc.sync.dma_start(wt[1], wt_d[1])
            nc.gpsimd.dma_start(enc[3], enc_d[3])
            nc.sync.dma_start(wt[2], wt_d[2])
            nc.sync.dma_start(wt[3], wt_d[3])
            nc.gpsimd.dma_start(thr[0], thr_d[0])
            nc.sync.dma_start(enc[4], enc_d[4])  # DVE bank ring, needed last
            # group 1 thresholds issued LAST: the scheduler then orders all
            # group-0 tail ops before group-1's, matching actual PSUM-bank
            # readiness (g1's banks finish ~1.4us after g0's); at runtime
            # this tiny ring still lands before g1's ew does
            nc.sync.dma_start(thr[1], thr_d[1])

            # ---- PSUM: per j-group, an ACT bank (320) and a DVE bank (192)
            # declared full-bank (512 f32) so no two tiles share a bank's
            # read port; only the leading AW/DW columns are used
            pwarm = ps.tile([128, 512], f32, name="pwarm")
            pL = [ps.tile([128, 512], f32, name=f"pL{g}") for g in range(2)]
            pR = [ps.tile([128, 512], f32, name=f"pR{g}") for g in range(2)]

            # PE warm-up transposes keep the clock ramping until data lands
            for _ in range(WARMUP):
                nc.tensor.transpose(pwarm[:, 0:128], identity, identity)

            # ew[j, b] = sum_k Ww[j,k] enc[b,k].  Both L banks (early
            # rings) are computed before the R banks so the ACT lane can
            # start while the R ring is still streaming.
            def series(pq, g, rings, base):
                lo = 0
                for q in rings:
                    for k in range(KC):
                        nc.tensor.matmul(
                            pq[:, lo:lo + RW[q]],
                            lhsT=wt[k // 2][:, k % 2, g, :],
                            rhs=enc[q][:, k, :],
                            start=(k == 0), stop=(k == KC - 1))
                    lo += RW[q]

            series(pL[0], 0, (0, 1, 2, 3), 0)
            series(pL[1], 1, (0, 1, 2, 3), 0)
            series(pR[0], 0, (4,), 0)
            series(pR[1], 1, (4,), 0)

            # ---- tail: ACT Sign(ew + nthr_i) {-1,0,1} (host: >= 0 -> +1);
            #            DVE (ew >= tpos_i) {1,0}     (host: > 0  -> +1)
            o_act = res.tile([128, 2, NB, AW], i8, name="o_act")
            o_dve = res.tile([128, 2, NB, DW], i8, name="o_dve")
            for g in range(2):
                for i in range(NB):
                    nc.scalar.activation(o_act[:, g, i, :], pL[g][:, 0:AW],
                                         AF.Sign, bias=thr[g][:, i:i + 1])
                    nc.vector.tensor_scalar(o_dve[:, g, i, :],
                                            pR[g][:, 0:DW],
                                            thr[g][:, 5 + i:6 + i], None,
                                            ALU.is_ge)
                nc.sync.dma_start(oa_d[:, g], o_act[:, g])
                nc.sync.dma_start(od_d[:, g], o_dve[:, g])

    nc.compile()
    return nc


def _get_nc():
    nc = _NC_CACHE.get("nc")
    if nc is None:
        nc = _build_nc()
        _NC_CACHE["nc"] = nc
    return nc


def _f16(a):
    return np.ascontiguousarray(a, dtype=np.float16)


def _chunkT(mat):
    # [H(k), F] -> [128, KC, F]: partition p holds k-chunk rows k*128+p
    F = mat.shape[1]
    return np.ascontiguousarray(mat.reshape(KC, 128, F).transpose(1, 0, 2))


def _numpy_fallback(enc, h, keys, Uw, Vw, Ww, prelu_a):
    gate = 1.0 / (1.0 + np.exp(-(enc @ (h + keys).T)))
    pre = (h @ Uw.T + keys @ Vw.T)[:, None, :] + (enc @ Ww.T)[None, :, :]
    cand = np.where(pre >= 0, pre, prelu_a * pre)
    new = h[:, None, :] + gate.T[:, None, :] * cand
    new = np.where(new == 0, np.float32(0.1), new)
    new = np.sign(new).astype(np.float32)
    return new.reshape(NB * B, H)


def kernel(features, states, Uw, Vw, Ww, keys, prelu_a):
    from concourse import bass_utils
    import os

    features = np.asarray(features)
    states = np.asarray(states, dtype=np.float32)
    Uw = np.asarray(Uw, dtype=np.float32)
    Vw = np.asarray(Vw, dtype=np.float32)
    Ww = np.asarray(Ww, dtype=np.float32)
    keys = np.asarray(keys, dtype=np.float32)
    prelu_a = np.asarray(prelu_a, dtype=np.float32)

    enc = np.ascontiguousarray(features[:, 0, :], dtype=np.float32)  # [B, H]
    h = states.reshape(NB, H)

    if np.any(prelu_a <= 0):
        # new is not monotone in ew for a <= 0; never hit in practice
        return _numpy_fallback(enc, h, keys, Uw, Vw, Ww, prelu_a)
    nc = _get_nc()

    # exact thresholds (float64) from the small operands
    e64 = enc.astype(np.float64)
    h64 = h.astype(np.float64)
    k64 = keys.astype(np.float64)
    z = e64 @ (h64 + k64).T                                   # [j, i]
    huv = Uw.astype(np.float64) @ h64.T + Vw.astype(np.float64) @ k64.T
    s = np.where(h64.T > 0, prelu_a.astype(np.float64)[:, None], 1.0)
    with np.errstate(over='ignore'):
        hos = h64.T / s
        nthr = huv + hos * (1.0 + np.exp(-z))
    nthr = np.clip(nthr, -1e30, 1e30).astype(np.float32)      # [H(j), NB]

    # enc.T fp16-single, chunked [128, KC, B]; each b-half feeds 4 cores
    e3 = _chunkT(_f16(enc.T))
    RO = np.cumsum([0] + RW)
    enc_halves = []
    for bh in range(2):
        eh = e3[:, :, bh * HB:(bh + 1) * HB]
        enc_halves.append({
            f"enc{q}": np.ascontiguousarray(eh[:, :, RO[q]:RO[q + 1]])
            for q in range(5)})

    in_maps = []
    for c in range(NCORES):
        jq, bh = c % NJ, c // NJ
        js = slice(jq * JS, (jq + 1) * JS)
        wtc = _chunkT(_f16(Ww[js].T)).reshape(128, KC, 2, 128)
        im = {**enc_halves[bh]}
        for g in range(2):
            jg = slice(jq * JS + g * 128, jq * JS + (g + 1) * 128)
            thrc = np.zeros((128, 16), dtype=np.float32)
            thrc[:, 0:5] = nthr[jg]
            thrc[:, 5:10] = -nthr[jg]
            im[f"thr{g}"] = thrc
        for p in range(4):
            im[f"wt{p}"] = np.ascontiguousarray(wtc[:, 2 * p:2 * p + 2])
        in_maps.append(im)

    trace = bool(int(os.environ.get("KERNEL_TRACE", "0")))
    res = bass_utils.run_bass_kernel_spmd(
        nc, in_maps, core_ids=list(range(NCORES)), trace=trace)
    kernel.last_result = res

    one = np.float32(1.0)
    neg = np.float32(-1.0)
    full = np.empty((NB, B, H), dtype=np.float32)
    ok = np.empty((NB, HB, 128), dtype=np.float32)
    for c in range(NCORES):
        jq, bh = c % NJ, c // NJ
        oa = res.results[c]["oa"]                  # [128, 2, NB, AW] int8
        od = res.results[c]["od"]                  # [128, 2, NB, DW] int8
        bs = slice(bh * HB, (bh + 1) * HB)
        for g in range(2):
            a = oa[:, g].transpose(1, 2, 0)        # [NB, AW, 128]
            d = od[:, g].transpose(1, 2, 0)        # [NB, DW, 128]
            ok[:, 0:AW] = np.where(a >= 0, one, neg)
            ok[:, AW:HB] = np.where(d > 0, one, neg)
            j0 = jq * JS + g * 128
            full[:, bs, j0:j0 + 128] = ok
    return full.reshape(NB * B, H)


# revision 30
# speedup vs baseline: 1.0012x; 1.0012x over previous
"""Trainium2 Bass kernel for nn_MemoryCell (scatter_memory), v5.

Full-input contract: kernel(**inputs) takes the complete (unsharded) numpy
inputs and returns the full [NB*B, H] output.

Math (B == H == 1024, NB == 5, T == 128):
    enc  = features[:, 0, :]                         # [B, H] - only slice used
    h    = states.reshape(NB, H)
    gate = sigmoid(enc @ (h + keys).T)               # [B, NB]
    pre  = (h @ Uw.T + keys @ Vw.T)[:, None, :] + (enc @ Ww.T)[None, :, :]
    cand = where(pre >= 0, pre, prelu_a * pre)
    new[i, b, j] = h[i, j] + gate[j, i] * cand[i, b, j]   # B==H broadcast quirk
    out  = sign(new) with exact zeros -> +1, reshaped [NB*B, H]

Because gate > 0 and (for prelu slope a > 0) new is monotone in ew =
enc @ Ww.T, each output element is a pure threshold test:

    sign(new[i, b, j]) = +1  iff  ew[j, b] + nthr[j, i] >= 0
    nthr = huv + (h / s) * (1 + exp(-z)),  s = a if h > 0 else 1

nthr is a tiny [H, NB] tensor: the host computes it exactly (float64) from
the small operands.  The device does the big work: stream enc (fp16), run
the ew matmul, apply one threshold compare per (block, j-group, bank).

Sharding is 2D: 4 j-shards (256 features) x 2 b-halves (512 batch), so
each core streams only HALF of enc (1.05 MB) + its Ww j-shard (0.52 MB)
fp16: 1.58 MB in, 0.65 MB out per core (vs 36 MB naive / 7.2 MB for the
prior fp16-hi/lo kernel).

Hardware notes baked into the structure (from perfetto traces):
  * PSUM bank reads serialize across engines per instruction, so the ACT
    and DVE tail lanes own disjoint PSUM banks; widths 320/192 balance
    ACT (0.83 ns/elem + ~160ns/op) vs DVE (1.04 ns/elem + ~220ns/op).
  * ACT and DVE lanes write separate SBUF output tiles (a shared tile
    serializes the writers through the framework's WAW ordering).
  * enc ships as 5 rings (4 x 80 cols for the ACT banks + 192 for the
    DVE banks) and Ww as 4 k-split rings: concurrent rings keep the DMA
    descriptor pipelines fed (a single ring is capped at ~230 GB/s by
    per-descriptor overhead), small rings complete first under the DMA
    engines' round-robin (so first-needed data is in small rings, the
    last-needed DVE ring is the big one), and first-wave descriptor
    gens issue from 3 engines in parallel (each dma_start costs ~650ns
    serialized per descriptor-gen path).
  * PE warm-up transposes bridge the DMA wait; the PE clock needs ~6us
    of continuous activity to reach full speed, so early ew matmuls run
    at the mid p-state and the warm-up just keeps the ramp going.
Measured 109 sign flips vs the 524-flip (2e-2 rel err) budget.
"""

import numpy as np

H = 1024
NB = 5
B = 1024
NCORES = 8
NJ = 4                    # j shards
JS = H // NJ              # 256 features per core (2 PE groups of 128)
HB = B // 2               # 512 batch columns per core
KC = H // 128             # 8 contraction chunks
AW = 320                  # tail columns on ACT per group (rest on DVE)
DW = HB - AW              # 192
RW = [80, 80, 80, 80, 192]  # enc rings: 4 small (ACT bank) + 1 (DVE)
WARMUP = 20

_NC_CACHE = {}


def _build_nc():
    from concourse import bacc, mybir
    import concourse.tile as tile
    from concourse.masks import make_identity

    f32 = mybir.dt.float32
    f16 = mybir.dt.float16
    i8 = mybir.dt.int8
    AF = mybir.ActivationFunctionType
    ALU = mybir.AluOpType

    nc = bacc.Bacc("TRN2", debug=False, num_devices=NCORES)

    # wt ships as 4 k-split rings so the small rings complete first under
    # the DMA engines' round-robin (completion order ~ ring size)
    wt_d = [nc.dram_tensor(f"wt{p}", [128, 2, 2, 128], f16,
                           kind="ExternalInput").ap() for p in range(4)]
    thr_d = [nc.dram_tensor(f"thr{g}", [128, 16], f32,
                            kind="ExternalInput").ap() for g in range(2)]
    enc_d = [nc.dram_tensor(f"enc{q}", [128, KC, RW[q]], f16,
                            kind="ExternalInput").ap() for q in range(5)]
    oa_d = nc.dram_tensor("oa", [128, 2, NB, AW], i8, kind="ExternalOutput").ap()
    od_d = nc.dram_tensor("od", [128, 2, NB, DW], i8, kind="ExternalOutput").ap()

    with tile.TileContext(nc) as tc:
        with (
            tc.tile_pool(name="res", bufs=1) as res,
            tc.tile_pool(name="ps", bufs=1, space="PSUM") as ps,
        ):
            # identity for PE warm-up FIRST: make_identity runs on gpsimd,
            # which must not be stuck behind its DMA descriptor gens
            identity = res.tile([128, 128], f32, name="identity")
            make_identity(nc, identity)

            # ---- input DMAs; first wave issues from 3 engines in parallel
            wt = [res.tile([128, 2, 2, 128], f16, name=f"wt{p}")
                  for p in range(4)]
            thr = [res.tile([128, 16], f32, name=f"thr{g}")
                   for g in range(2)]
            enc = [res.tile([128, KC, RW[q]], f16, name=f"enc{q}")
                   for q in range(5)]
            nc.sync.dma_start(wt[0], wt_d[0])
            nc.scalar.dma_start(enc[0], enc_d[0])
            nc.scalar.dma_start(enc[1], enc_d[1])
            nc.gpsimd.dma_start(enc[2], enc_d[2])
            nc.sync.dma_start(wt[1], wt_d[1])
            nc.gpsimd.dma_start(enc[3], enc_d[3])
            nc.sync.dma_start(wt[2], wt_d[2])
            nc.sync.dma_start(wt[3], wt_d[3])
            nc.gpsimd.dma_start(thr[0], thr_d[0])
            nc.sync.dma_start(enc[4], enc_d[4])  # DVE bank ring, needed last
            # group 1 thresholds issued LAST: the scheduler then orders all
            # group-0 tail ops before group-1's, matching actual PSUM-bank
            # readiness (g1's banks finish ~1.4us after g0's); at runtime
            # this tiny ring still lands before g1's ew does
            nc.sync.dma_start(thr[1], thr_d[1])

            # ---- PSUM: per j-group, an ACT bank (320) and a DVE bank (192)
            # declared full-bank (512 f32) so no two tiles share a bank's
            # read port; only the leading AW/DW columns are used
            pwarm = ps.tile([128, 512], f32, name="pwarm")
            pL = [ps.tile([128, 512], f32, name=f"pL{g}") for g in range(2)]
            pR = [ps.tile([128, 512], f32, name=f"pR{g}") for g in range(2)]

            # PE warm-up transposes keep the clock ramping until data lands
            for _ in range(WARMUP):
                nc.tensor.transpose(pwarm[:, 0:128], identity, identity)

            # ew[j, b] = sum_k Ww[j,k] enc[b,k].  Both L banks (early
            # rings) are computed before the R banks so the ACT lane can
            # start while the R ring is still streaming.
            def series(pq, g, rings, base):
                lo = 0
                for q in rings:
                    for k in range(KC):
                        nc.tensor.matmul(
                            pq[:, lo:lo + RW[q]],
                            lhsT=wt[k // 2][:, k % 2, g, :],
                            rhs=enc[q][:, k, :],
                            start=(k == 0), stop=(k == KC - 1))
                    lo += RW[q]

            series(pL[0], 0, (0, 1, 2, 3), 0)
            series(pL[1], 1, (0, 1, 2, 3), 0)
            series(pR[0], 0, (4,), 0)
            series(pR[1], 1, (4,), 0)

            # ---- tail: ACT Sign(ew + nthr_i) {-1,0,1} (host: >= 0 -> +1);
            #            DVE (ew >= tpos_i) {1,0}     (host: > 0  -> +1)
            o_act = res.tile([128, 2, NB, AW], i8, name="o_act")
            o_dve = res.tile([128, 2, NB, DW], i8, name="o_dve")
            for g in range(2):
                for i in range(NB):
                    nc.scalar.activation(o_act[:, g, i, :], pL[g][:, 0:AW],
                                         AF.Sign, bias=thr[g][:, i:i + 1])
                    nc.vector.tensor_scalar(o_dve[:, g, i, :],
                                            pR[g][:, 0:DW],
                                            thr[g][:, 5 + i:6 + i], None,
                                            ALU.is_ge)
            # out gens spread across engines: 4 serialized gens on sync
            # would delay the last transfer by ~1.5us
            nc.sync.dma_start(oa_d[:, 0], o_act[:, 0])
            nc.gpsimd.dma_start(od_d[:, 0], o_dve[:, 0])
            nc.scalar.dma_start(oa_d[:, 1], o_act[:, 1])
            nc.gpsimd.dma_start(od_d[:, 1], o_dve[:, 1])

    nc.compile()
    return nc


def _get_nc():
    nc = _NC_CACHE.get("nc")
    if nc is None:
        nc = _build_nc()
        _NC_CACHE["nc"] = nc
    return nc


def _f16(a):
    return np.ascontiguousarray(a, dtype=np.float16)


def _chunkT(mat):
    # [H(k), F] -> [128, KC, F]: partition p holds k-chunk rows k*128+p
    F = mat.shape[1]
    return np.ascontiguousarray(mat.reshape(KC, 128, F).transpose(1, 0, 2))


def _numpy_fallback(enc, h, keys, Uw, Vw, Ww, prelu_a):
    gate = 1.0 / (1.0 + np.exp(-(enc @ (h + keys).T)))
    pre = (h @ Uw.T + keys @ Vw.T)[:, None, :] + (enc @ Ww.T)[None, :, :]
    cand = np.where(pre >= 0, pre, prelu_a * pre)
    new = h[:, None, :] + gate.T[:, None, :] * cand
    new = np.where(new == 0, np.float32(0.1), new)
    new = np.sign(new).astype(np.float32)
    return new.reshape(NB * B, H)


def kernel(features, states, Uw, Vw, Ww, keys, prelu_a):
    from concourse import bass_utils
    import os

    features = np.asarray(features)
    states = np.asarray(states, dtype=np.float32)
    Uw = np.asarray(Uw, dtype=np.float32)
    Vw = np.asarray(Vw, dtype=np.float32)
    Ww = np.asarray(Ww, dtype=np.float32)
    keys = np.asarray(keys, dtype=np.float32)
    prelu_a = np.asarray(prelu_a, dtype=np.float32)

    enc = np.ascontiguousarray(features[:, 0, :], dtype=np.float32)  # [B, H]
    h = states.reshape(NB, H)

    if np.any(prelu_a <= 0):
        # new is not monotone in ew for a <= 0; never hit in practice
        return _numpy_fallback(enc, h, keys, Uw, Vw, Ww, prelu_a)
    nc = _get_nc()

    # exact thresholds (float64) from the small operands
    e64 = enc.astype(np.float64)
    h64 = h.astype(np.float64)
    k64 = keys.astype(np.float64)
    z = e64 @ (h64 + k64).T                                   # [j, i]
    huv = Uw.astype(np.float64) @ h64.T + Vw.astype(np.float64) @ k64.T
    s = np.where(h64.T > 0, prelu_a.astype(np.float64)[:, None], 1.0)
    with np.errstate(over='ignore'):
        hos = h64.T / s
        nthr = huv + hos * (1.0 + np.exp(-z))
    nthr = np.clip(nthr, -1e30, 1e30).astype(np.float32)      # [H(j), NB]

    # enc.T fp16-single, chunked [128, KC, B]; each b-half feeds 4 cores
    e3 = _chunkT(_f16(enc.T))
    RO = np.cumsum([0] + RW)
    enc_halves = []
    for bh in range(2):
        eh = e3[:, :, bh * HB:(bh + 1) * HB]
        enc_halves.append({
            f"enc{q}": np.ascontiguousarray(eh[:, :, RO[q]:RO[q + 1]])
            for q in range(5)})

    in_maps = []
    for c in range(NCORES):
        jq, bh = c % NJ, c // NJ
        js = slice(jq * JS, (jq + 1) * JS)
        wtc = _chunkT(_f16(Ww[js].T)).reshape(128, KC, 2, 128)
        im = {**enc_halves[bh]}
        for g in range(2):
            jg = slice(jq * JS + g * 128, jq * JS + (g + 1) * 128)
            thrc = np.zeros((128, 16), dtype=np.float32)
            thrc[:, 0:5] = nthr[jg]
            thrc[:, 5:10] = -nthr[jg]
            im[f"thr{g}"] = thrc
        for p in range(4):
            im[f"wt{p}"] = np.ascontiguousarray(wtc[:, 2 * p:2 * p + 2])
        in_maps.append(im)

    trace = bool(int(os.environ.get("KERNEL_TRACE", "0")))
    res = bass_utils.run_bass_kernel_spmd(
        nc, in_maps, core_ids=list(range(NCORES)), trace=trace)
    kernel.last_result = res

    one = np.float32(1.0)
    neg = np.float32(-1.0)
    full = np.empty((NB, B, H), dtype=np.float32)
    ok = np.empty((NB, HB, 128), dtype=np.float32)
    for c in range(NCORES):
        jq, bh = c % NJ, c // NJ
        oa = res.results[c]["oa"]                  # [128, 2, NB, AW] int8
        od = res.results[c]["od"]                  # [128, 2, NB, DW] int8
        bs = slice(bh * HB, (bh + 1) * HB)
        for g in range(2):
            a = oa[:, g].transpose(1, 2, 0)        # [NB, AW, 128]
            d = od[:, g].transpose(1, 2, 0)        # [NB, DW, 128]
            ok[:, 0:AW] = np.where(a >= 0, one, neg)
            ok[:, AW:HB] = np.where(d > 0, one, neg)
            j0 = jq * JS + g * 128
            full[:, bs, j0:j0 + 128] = ok
    return full.reshape(NB * B, H)


# revision 31
# speedup vs baseline: 1.0377x; 1.0364x over previous
"""Trainium2 Bass kernel for nn_MemoryCell (scatter_memory), v5.

Full-input contract: kernel(**inputs) takes the complete (unsharded) numpy
inputs and returns the full [NB*B, H] output.

Math (B == H == 1024, NB == 5, T == 128):
    enc  = features[:, 0, :]                         # [B, H] - only slice used
    h    = states.reshape(NB, H)
    gate = sigmoid(enc @ (h + keys).T)               # [B, NB]
    pre  = (h @ Uw.T + keys @ Vw.T)[:, None, :] + (enc @ Ww.T)[None, :, :]
    cand = where(pre >= 0, pre, prelu_a * pre)
    new[i, b, j] = h[i, j] + gate[j, i] * cand[i, b, j]   # B==H broadcast quirk
    out  = sign(new) with exact zeros -> +1, reshaped [NB*B, H]

Because gate > 0 and (for prelu slope a > 0) new is monotone in ew =
enc @ Ww.T, each output element is a pure threshold test:

    sign(new[i, b, j]) = +1  iff  ew[j, b] + nthr[j, i] >= 0
    nthr = huv + (h / s) * (1 + exp(-z)),  s = a if h > 0 else 1

nthr is a tiny [H, NB] tensor: the host computes it exactly (float64) from
the small operands.  The device does the big work: stream enc (fp16), run
the ew matmul, apply one threshold compare per (block, j-group, bank).

Sharding is 2D: 4 j-shards (256 features) x 2 b-halves (512 batch), so
each core streams only HALF of enc (1.05 MB) + its Ww j-shard (0.52 MB)
fp16: 1.58 MB in, 0.65 MB out per core (vs 36 MB naive / 7.2 MB for the
prior fp16-hi/lo kernel).

Hardware notes baked into the structure (from perfetto traces):
  * PSUM bank reads serialize across engines per instruction, so the ACT
    and DVE tail lanes own disjoint PSUM banks; widths 320/192 balance
    ACT (0.83 ns/elem + ~160ns/op) vs DVE (1.04 ns/elem + ~220ns/op).
  * ACT and DVE lanes write separate SBUF output tiles (a shared tile
    serializes the writers through the framework's WAW ordering).
  * enc ships as 5 rings (4 x 80 cols for the ACT banks + 192 for the
    DVE banks) and Ww as 4 k-split rings: concurrent rings keep the DMA
    descriptor pipelines fed (a single ring is capped at ~230 GB/s by
    per-descriptor overhead), small rings complete first under the DMA
    engines' round-robin (so first-needed data is in small rings, the
    last-needed DVE ring is the big one), and first-wave descriptor
    gens issue from 3 engines in parallel (each dma_start costs ~650ns
    serialized per descriptor-gen path).
  * PE warm-up transposes bridge the DMA wait; the PE clock needs ~6us
    of continuous activity to reach full speed, so early ew matmuls run
    at the mid p-state and the warm-up just keeps the ramp going.
Measured 109 sign flips vs the 524-flip (2e-2 rel err) budget.
"""

import numpy as np

H = 1024
NB = 5
B = 1024
NCORES = 8
NJ = 4                    # j shards
JS = H // NJ              # 256 features per core (2 PE groups of 128)
HB = B // 2               # 512 batch columns per core
KC = H // 128             # 8 contraction chunks
AW = 368                  # tail columns on ACT per group (rest on DVE)
DW = HB - AW              # 192
RW = [92, 92, 92, 92, 144]  # enc rings: 4 small (ACT bank) + 1 (DVE)
WARMUP = 20

_NC_CACHE = {}


def _build_nc():
    from concourse import bacc, mybir
    import concourse.tile as tile
    from concourse.masks import make_identity

    f32 = mybir.dt.float32
    f16 = mybir.dt.float16
    i8 = mybir.dt.int8
    AF = mybir.ActivationFunctionType
    ALU = mybir.AluOpType

    nc = bacc.Bacc("TRN2", debug=False, num_devices=NCORES)

    # wt ships as 4 k-split rings so the small rings complete first under
    # the DMA engines' round-robin (completion order ~ ring size)
    wt_d = [nc.dram_tensor(f"wt{p}", [128, 2, 2, 128], f16,
                           kind="ExternalInput").ap() for p in range(4)]
    thr_d = [nc.dram_tensor(f"thr{g}", [128, 16], f32,
                            kind="ExternalInput").ap() for g in range(2)]
    enc_d = [nc.dram_tensor(f"enc{q}", [128, KC, RW[q]], f16,
                            kind="ExternalInput").ap() for q in range(5)]
    oa_d = nc.dram_tensor("oa", [128, 2, NB, AW], i8, kind="ExternalOutput").ap()
    od_d = nc.dram_tensor("od", [128, 2, NB, DW], i8, kind="ExternalOutput").ap()

    with tile.TileContext(nc) as tc:
        with (
            tc.tile_pool(name="res", bufs=1) as res,
            tc.tile_pool(name="ps", bufs=1, space="PSUM") as ps,
        ):
            # identity for PE warm-up FIRST: make_identity runs on gpsimd,
            # which must not be stuck behind its DMA descriptor gens
            identity = res.tile([128, 128], f32, name="identity")
            make_identity(nc, identity)

            # ---- input DMAs; first wave issues from 3 engines in parallel
            wt = [res.tile([128, 2, 2, 128], f16, name=f"wt{p}")
                  for p in range(4)]
            thr = [res.tile([128, 16], f32, name=f"thr{g}")
                   for g in range(2)]
            enc = [res.tile([128, KC, RW[q]], f16, name=f"enc{q}")
                   for q in range(5)]
            nc.sync.dma_start(wt[0], wt_d[0])
            nc.scalar.dma_start(enc[0], enc_d[0])
            nc.scalar.dma_start(enc[1], enc_d[1])
            nc.gpsimd.dma_start(enc[2], enc_d[2])
            nc.sync.dma_start(wt[1], wt_d[1])
            nc.gpsimd.dma_start(enc[3], enc_d[3])
            nc.sync.dma_start(wt[2], wt_d[2])
            nc.sync.dma_start(wt[3], wt_d[3])
            nc.gpsimd.dma_start(thr[0], thr_d[0])
            nc.sync.dma_start(enc[4], enc_d[4])  # DVE bank ring, needed last
            # group 1 thresholds issued LAST: the scheduler then orders all
            # group-0 tail ops before group-1's, matching actual PSUM-bank
            # readiness (g1's banks finish ~1.4us after g0's); at runtime
            # this tiny ring still lands before g1's ew does
            nc.sync.dma_start(thr[1], thr_d[1])

            # ---- PSUM: per j-group, an ACT bank (320) and a DVE bank (192)
            # declared full-bank (512 f32) so no two tiles share a bank's
            # read port; only the leading AW/DW columns are used
            pwarm = ps.tile([128, 512], f32, name="pwarm")
            pL = [ps.tile([128, 512], f32, name=f"pL{g}") for g in range(2)]
            pR = [ps.tile([128, 512], f32, name=f"pR{g}") for g in range(2)]

            # PE warm-up transposes keep the clock ramping until data lands
            for _ in range(WARMUP):
                nc.tensor.transpose(pwarm[:, 0:128], identity, identity)

            # ew[j, b] = sum_k Ww[j,k] enc[b,k].  Both L banks (early
            # rings) are computed before the R banks so the ACT lane can
            # start while the R ring is still streaming.
            def series(pq, g, rings, base):
                lo = 0
                for q in rings:
                    for k in range(KC):
                        nc.tensor.matmul(
                            pq[:, lo:lo + RW[q]],
                            lhsT=wt[k // 2][:, k % 2, g, :],
                            rhs=enc[q][:, k, :],
                            start=(k == 0), stop=(k == KC - 1))
                    lo += RW[q]

            series(pL[0], 0, (0, 1, 2, 3), 0)
            series(pL[1], 1, (0, 1, 2, 3), 0)
            series(pR[0], 0, (4,), 0)
            series(pR[1], 1, (4,), 0)

            # ---- tail: ACT Sign(ew + nthr_i) {-1,0,1} (host: >= 0 -> +1);
            #            DVE (ew >= tpos_i) {1,0}     (host: > 0  -> +1)
            o_act = res.tile([128, 2, NB, AW], i8, name="o_act")
            o_dve = res.tile([128, 2, NB, DW], i8, name="o_dve")
            for g in range(2):
                for i in range(NB):
                    nc.scalar.activation(o_act[:, g, i, :], pL[g][:, 0:AW],
                                         AF.Sign, bias=thr[g][:, i:i + 1])
                    nc.vector.tensor_scalar(o_dve[:, g, i, :],
                                            pR[g][:, 0:DW],
                                            thr[g][:, 5 + i:6 + i], None,
                                            ALU.is_ge)
            # out gens spread across engines: 4 serialized gens on sync
            # would delay the last transfer by ~1.5us
            nc.sync.dma_start(oa_d[:, 0], o_act[:, 0])
            nc.gpsimd.dma_start(od_d[:, 0], o_dve[:, 0])
            nc.scalar.dma_start(oa_d[:, 1], o_act[:, 1])
            # the last-finishing output rides sync's faster HWDGE gen path
            nc.sync.dma_start(od_d[:, 1], o_dve[:, 1])

    nc.compile()
    return nc


def _get_nc():
    nc = _NC_CACHE.get("nc")
    if nc is None:
        nc = _build_nc()
        _NC_CACHE["nc"] = nc
    return nc


def _f16(a):
    return np.ascontiguousarray(a, dtype=np.float16)


def _chunkT(mat):
    # [H(k), F] -> [128, KC, F]: partition p holds k-chunk rows k*128+p
    F = mat.shape[1]
    return np.ascontiguousarray(mat.reshape(KC, 128, F).transpose(1, 0, 2))


def _numpy_fallback(enc, h, keys, Uw, Vw, Ww, prelu_a):
    gate = 1.0 / (1.0 + np.exp(-(enc @ (h + keys).T)))
    pre = (h @ Uw.T + keys @ Vw.T)[:, None, :] + (enc @ Ww.T)[None, :, :]
    cand = np.where(pre >= 0, pre, prelu_a * pre)
    new = h[:, None, :] + gate.T[:, None, :] * cand
    new = np.where(new == 0, np.float32(0.1), new)
    new = np.sign(new).astype(np.float32)
    return new.reshape(NB * B, H)


def kernel(features, states, Uw, Vw, Ww, keys, prelu_a):
    from concourse import bass_utils
    import os

    features = np.asarray(features)
    states = np.asarray(states, dtype=np.float32)
    Uw = np.asarray(Uw, dtype=np.float32)
    Vw = np.asarray(Vw, dtype=np.float32)
    Ww = np.asarray(Ww, dtype=np.float32)
    keys = np.asarray(keys, dtype=np.float32)
    prelu_a = np.asarray(prelu_a, dtype=np.float32)

    enc = np.ascontiguousarray(features[:, 0, :], dtype=np.float32)  # [B, H]
    h = states.reshape(NB, H)

    if np.any(prelu_a <= 0):
        # new is not monotone in ew for a <= 0; never hit in practice
        return _numpy_fallback(enc, h, keys, Uw, Vw, Ww, prelu_a)
    nc = _get_nc()

    # exact thresholds (float64) from the small operands
    e64 = enc.astype(np.float64)
    h64 = h.astype(np.float64)
    k64 = keys.astype(np.float64)
    z = e64 @ (h64 + k64).T                                   # [j, i]
    huv = Uw.astype(np.float64) @ h64.T + Vw.astype(np.float64) @ k64.T
    s = np.where(h64.T > 0, prelu_a.astype(np.float64)[:, None], 1.0)
    with np.errstate(over='ignore'):
        hos = h64.T / s
        nthr = huv + hos * (1.0 + np.exp(-z))
    nthr = np.clip(nthr, -1e30, 1e30).astype(np.float32)      # [H(j), NB]

    # enc.T fp16-single, chunked [128, KC, B]; each b-half feeds 4 cores
    e3 = _chunkT(_f16(enc.T))
    RO = np.cumsum([0] + RW)
    enc_halves = []
    for bh in range(2):
        eh = e3[:, :, bh * HB:(bh + 1) * HB]
        enc_halves.append({
            f"enc{q}": np.ascontiguousarray(eh[:, :, RO[q]:RO[q + 1]])
            for q in range(5)})

    in_maps = []
    for c in range(NCORES):
        jq, bh = c % NJ, c // NJ
        js = slice(jq * JS, (jq + 1) * JS)
        wtc = _chunkT(_f16(Ww[js].T)).reshape(128, KC, 2, 128)
        im = {**enc_halves[bh]}
        for g in range(2):
            jg = slice(jq * JS + g * 128, jq * JS + (g + 1) * 128)
            thrc = np.zeros((128, 16), dtype=np.float32)
            thrc[:, 0:5] = nthr[jg]
            thrc[:, 5:10] = -nthr[jg]
            im[f"thr{g}"] = thrc
        for p in range(4):
            im[f"wt{p}"] = np.ascontiguousarray(wtc[:, 2 * p:2 * p + 2])
        in_maps.append(im)

    trace = bool(int(os.environ.get("KERNEL_TRACE", "0")))
    res = bass_utils.run_bass_kernel_spmd(
        nc, in_maps, core_ids=list(range(NCORES)), trace=trace)
    kernel.last_result = res

    one = np.float32(1.0)
    neg = np.float32(-1.0)
    full = np.empty((NB, B, H), dtype=np.float32)
    ok = np.empty((NB, HB, 128), dtype=np.float32)
    for c in range(NCORES):
        jq, bh = c % NJ, c // NJ
        oa = res.results[c]["oa"]                  # [128, 2, NB, AW] int8
        od = res.results[c]["od"]                  # [128, 2, NB, DW] int8
        bs = slice(bh * HB, (bh + 1) * HB)
        for g in range(2):
            a = oa[:, g].transpose(1, 2, 0)        # [NB, AW, 128]
            d = od[:, g].transpose(1, 2, 0)        # [NB, DW, 128]
            ok[:, 0:AW] = np.where(a >= 0, one, neg)
            ok[:, AW:HB] = np.where(d > 0, one, neg)
            j0 = jq * JS + g * 128
            full[:, bs, j0:j0 + 128] = ok
    return full.reshape(NB * B, H)


# revision 32
# speedup vs baseline: 1.0393x; 1.0015x over previous
"""Trainium2 Bass kernel for nn_MemoryCell (scatter_memory), v5.

Full-input contract: kernel(**inputs) takes the complete (unsharded) numpy
inputs and returns the full [NB*B, H] output.

Math (B == H == 1024, NB == 5, T == 128):
    enc  = features[:, 0, :]                         # [B, H] - only slice used
    h    = states.reshape(NB, H)
    gate = sigmoid(enc @ (h + keys).T)               # [B, NB]
    pre  = (h @ Uw.T + keys @ Vw.T)[:, None, :] + (enc @ Ww.T)[None, :, :]
    cand = where(pre >= 0, pre, prelu_a * pre)
    new[i, b, j] = h[i, j] + gate[j, i] * cand[i, b, j]   # B==H broadcast quirk
    out  = sign(new) with exact zeros -> +1, reshaped [NB*B, H]

Because gate > 0 and (for prelu slope a > 0) new is monotone in ew =
enc @ Ww.T, each output element is a pure threshold test:

    sign(new[i, b, j]) = +1  iff  ew[j, b] + nthr[j, i] >= 0
    nthr = huv + (h / s) * (1 + exp(-z)),  s = a if h > 0 else 1

nthr is a tiny [H, NB] tensor: the host computes it exactly (float64) from
the small operands.  The device does the big work: stream enc (fp16), run
the ew matmul, apply one threshold compare per (block, j-group, bank).

Sharding is 2D: 4 j-shards (256 features) x 2 b-halves (512 batch), so
each core streams only HALF of enc (1.05 MB) + its Ww j-shard (0.52 MB)
fp16: 1.58 MB in, 0.65 MB out per core (vs 36 MB naive / 7.2 MB for the
prior fp16-hi/lo kernel).

Hardware notes baked into the structure (from perfetto traces):
  * PSUM bank reads serialize across engines per instruction, so the ACT
    and DVE tail lanes own disjoint PSUM banks; widths 368/144 balance
    ACT (0.83 ns/elem + ~160ns/op, early start) against DVE (1.04
    ns/elem, start pinned late by the last enc ring).
  * ACT and DVE lanes write separate SBUF output tiles (a shared tile
    serializes the writers through the framework's WAW ordering).
  * enc ships as 5 rings (4 x 92 cols for the ACT banks + 144 for the
    DVE banks) and Ww as 4 k-split rings: concurrent rings keep the DMA
    descriptor pipelines fed (a single ring is capped at ~230 GB/s by
    per-descriptor overhead), small rings complete first under the DMA
    engines' round-robin (so first-needed data is in small rings, the
    last-needed DVE ring is the big one), and first-wave descriptor
    gens issue from 3 engines in parallel (each dma_start costs ~650ns
    serialized per descriptor-gen path).
  * PE warm-up transposes bridge the DMA wait; the PE clock needs ~6us
    of continuous activity to reach full speed, so early ew matmuls run
    at the mid p-state and the warm-up just keeps the ramp going.
Measured 109 sign flips vs the 524-flip (2e-2 rel err) budget.
"""

import numpy as np

H = 1024
NB = 5
B = 1024
NCORES = 8
NJ = 4                    # j shards
JS = H // NJ              # 256 features per core (2 PE groups of 128)
HB = B // 2               # 512 batch columns per core
KC = H // 128             # 8 contraction chunks
AW = 368                  # tail columns on ACT per group (rest on DVE)
DW = HB - AW              # 144
RW = [92, 92, 92, 92, 144]  # enc rings: 4 small (ACT bank) + 1 (DVE)
WARMUP = 20

_NC_CACHE = {}


def _build_nc():
    from concourse import bacc, mybir
    import concourse.tile as tile
    from concourse.masks import make_identity

    f32 = mybir.dt.float32
    f16 = mybir.dt.float16
    i8 = mybir.dt.int8
    AF = mybir.ActivationFunctionType
    ALU = mybir.AluOpType

    nc = bacc.Bacc("TRN2", debug=False, num_devices=NCORES)

    # wt ships as 4 k-split rings so the small rings complete first under
    # the DMA engines' round-robin (completion order ~ ring size)
    wt_d = [nc.dram_tensor(f"wt{p}", [128, 2, 2, 128], f16,
                           kind="ExternalInput").ap() for p in range(4)]
    thr_d = [nc.dram_tensor(f"thr{g}", [128, 16], f32,
                            kind="ExternalInput").ap() for g in range(2)]
    enc_d = [nc.dram_tensor(f"enc{q}", [128, KC, RW[q]], f16,
                            kind="ExternalInput").ap() for q in range(5)]
    oa_d = nc.dram_tensor("oa", [128, 2, NB, AW], i8, kind="ExternalOutput").ap()
    od_d = nc.dram_tensor("od", [128, 2, NB, DW], i8, kind="ExternalOutput").ap()

    with tile.TileContext(nc) as tc:
        with (
            tc.tile_pool(name="res", bufs=1) as res,
            tc.tile_pool(name="ps", bufs=1, space="PSUM") as ps,
        ):
            # identity for PE warm-up FIRST: make_identity runs on gpsimd,
            # which must not be stuck behind its DMA descriptor gens
            identity = res.tile([128, 128], f32, name="identity")
            make_identity(nc, identity)

            # ---- input DMAs; first wave issues from 3 engines in parallel
            wt = [res.tile([128, 2, 2, 128], f16, name=f"wt{p}")
                  for p in range(4)]
            thr = [res.tile([128, 16], f32, name=f"thr{g}")
                   for g in range(2)]
            enc = [res.tile([128, KC, RW[q]], f16, name=f"enc{q}")
                   for q in range(5)]
            nc.sync.dma_start(wt[0], wt_d[0])
            nc.scalar.dma_start(enc[0], enc_d[0])
            nc.scalar.dma_start(enc[1], enc_d[1])
            nc.gpsimd.dma_start(enc[2], enc_d[2])
            nc.sync.dma_start(wt[1], wt_d[1])
            nc.gpsimd.dma_start(enc[3], enc_d[3])
            nc.sync.dma_start(wt[2], wt_d[2])
            nc.sync.dma_start(wt[3], wt_d[3])
            nc.gpsimd.dma_start(thr[0], thr_d[0])
            nc.sync.dma_start(enc[4], enc_d[4])  # DVE bank ring, needed last
            # group 1 thresholds issued LAST: the scheduler then orders all
            # group-0 tail ops before group-1's, matching actual PSUM-bank
            # readiness (g1's banks finish ~1.4us after g0's); at runtime
            # this tiny ring still lands before g1's ew does
            nc.sync.dma_start(thr[1], thr_d[1])

            # ---- PSUM: per j-group, an ACT bank (AW) and a DVE bank (DW)
            # declared full-bank (512 f32) so no two tiles share a bank's
            # read port; only the leading AW/DW columns are used
            pwarm = ps.tile([128, 512], f32, name="pwarm")
            pL = [ps.tile([128, 512], f32, name=f"pL{g}") for g in range(2)]
            pR = [ps.tile([128, 512], f32, name=f"pR{g}") for g in range(2)]

            # PE warm-up transposes keep the clock ramping until data lands
            for _ in range(WARMUP):
                nc.tensor.transpose(pwarm[:, 0:128], identity, identity)

            # ew[j, b] = sum_k Ww[j,k] enc[b,k].  Both L banks (early
            # rings) are computed before the R banks so the ACT lane can
            # start while the R ring is still streaming.
            def series(pq, g, rings, base):
                lo = 0
                for q in rings:
                    for k in range(KC):
                        nc.tensor.matmul(
                            pq[:, lo:lo + RW[q]],
                            lhsT=wt[k // 2][:, k % 2, g, :],
                            rhs=enc[q][:, k, :],
                            start=(k == 0), stop=(k == KC - 1))
                    lo += RW[q]

            series(pL[0], 0, (0, 1, 2, 3), 0)
            series(pL[1], 1, (0, 1, 2, 3), 0)
            series(pR[0], 0, (4,), 0)
            series(pR[1], 1, (4,), 0)

            # ---- tail: ACT Sign(ew + nthr_i) {-1,0,1} (host: >= 0 -> +1);
            #            DVE (ew >= tpos_i) {1,0}     (host: > 0  -> +1)
            o_act = res.tile([128, 2, NB, AW], i8, name="o_act")
            o_dve = res.tile([128, 2, NB, DW], i8, name="o_dve")
            for g in range(2):
                for i in range(NB):
                    nc.scalar.activation(o_act[:, g, i, :], pL[g][:, 0:AW],
                                         AF.Sign, bias=thr[g][:, i:i + 1])
                    nc.vector.tensor_scalar(o_dve[:, g, i, :],
                                            pR[g][:, 0:DW],
                                            thr[g][:, 5 + i:6 + i], None,
                                            ALU.is_ge)
            # out gens spread across engines: 4 serialized gens on sync
            # would delay the last transfer by ~1.5us
            nc.sync.dma_start(oa_d[:, 0], o_act[:, 0])
            nc.gpsimd.dma_start(od_d[:, 0], o_dve[:, 0])
            nc.scalar.dma_start(oa_d[:, 1], o_act[:, 1])
            # the last-finishing output rides sync's faster HWDGE gen path
            nc.sync.dma_start(od_d[:, 1], o_dve[:, 1])

    nc.compile()
    return nc


def _get_nc():
    nc = _NC_CACHE.get("nc")
    if nc is None:
        nc = _build_nc()
        _NC_CACHE["nc"] = nc
    return nc


def _f16(a):
    return np.ascontiguousarray(a, dtype=np.float16)


def _chunkT(mat):
    # [H(k), F] -> [128, KC, F]: partition p holds k-chunk rows k*128+p
    F = mat.shape[1]
    return np.ascontiguousarray(mat.reshape(KC, 128, F).transpose(1, 0, 2))


def _numpy_fallback(enc, h, keys, Uw, Vw, Ww, prelu_a):
    gate = 1.0 / (1.0 + np.exp(-(enc @ (h + keys).T)))
    pre = (h @ Uw.T + keys @ Vw.T)[:, None, :] + (enc @ Ww.T)[None, :, :]
    cand = np.where(pre >= 0, pre, prelu_a * pre)
    new = h[:, None, :] + gate.T[:, None, :] * cand
    new = np.where(new == 0, np.float32(0.1), new)
    new = np.sign(new).astype(np.float32)
    return new.reshape(NB * B, H)


def kernel(features, states, Uw, Vw, Ww, keys, prelu_a):
    from concourse import bass_utils
    import os

    features = np.asarray(features)
    states = np.asarray(states, dtype=np.float32)
    Uw = np.asarray(Uw, dtype=np.float32)
    Vw = np.asarray(Vw, dtype=np.float32)
    Ww = np.asarray(Ww, dtype=np.float32)
    keys = np.asarray(keys, dtype=np.float32)
    prelu_a = np.asarray(prelu_a, dtype=np.float32)

    enc = np.ascontiguousarray(features[:, 0, :], dtype=np.float32)  # [B, H]
    h = states.reshape(NB, H)

    if np.any(prelu_a <= 0):
        # new is not monotone in ew for a <= 0; never hit in practice
        return _numpy_fallback(enc, h, keys, Uw, Vw, Ww, prelu_a)
    nc = _get_nc()

    # exact thresholds (float64) from the small operands
    e64 = enc.astype(np.float64)
    h64 = h.astype(np.float64)
    k64 = keys.astype(np.float64)
    z = e64 @ (h64 + k64).T                                   # [j, i]
    huv = Uw.astype(np.float64) @ h64.T + Vw.astype(np.float64) @ k64.T
    s = np.where(h64.T > 0, prelu_a.astype(np.float64)[:, None], 1.0)
    with np.errstate(over='ignore'):
        hos = h64.T / s
        nthr = huv + hos * (1.0 + np.exp(-z))
    nthr = np.clip(nthr, -1e30, 1e30).astype(np.float32)      # [H(j), NB]

    # enc.T fp16-single, chunked [128, KC, B]; each b-half feeds 4 cores
    e3 = _chunkT(_f16(enc.T))
    RO = np.cumsum([0] + RW)
    enc_halves = []
    for bh in range(2):
        eh = e3[:, :, bh * HB:(bh + 1) * HB]
        enc_halves.append({
            f"enc{q}": np.ascontiguousarray(eh[:, :, RO[q]:RO[q + 1]])
            for q in range(5)})

    in_maps = []
    for c in range(NCORES):
        jq, bh = c % NJ, c // NJ
        js = slice(jq * JS, (jq + 1) * JS)
        wtc = _chunkT(_f16(Ww[js].T)).reshape(128, KC, 2, 128)
        im = {**enc_halves[bh]}
        for g in range(2):
            jg = slice(jq * JS + g * 128, jq * JS + (g + 1) * 128)
            thrc = np.zeros((128, 16), dtype=np.float32)
            thrc[:, 0:5] = nthr[jg]
            thrc[:, 5:10] = -nthr[jg]
            im[f"thr{g}"] = thrc
        for p in range(4):
            im[f"wt{p}"] = np.ascontiguousarray(wtc[:, 2 * p:2 * p + 2])
        in_maps.append(im)

    trace = bool(int(os.environ.get("KERNEL_TRACE", "0")))
    res = bass_utils.run_bass_kernel_spmd(
        nc, in_maps, core_ids=list(range(NCORES)), trace=trace)
    kernel.last_result = res

    one = np.float32(1.0)
    neg = np.float32(-1.0)
    full = np.empty((NB, B, H), dtype=np.float32)
    ok = np.empty((NB, HB, 128), dtype=np.float32)
    for c in range(NCORES):
        jq, bh = c % NJ, c // NJ
        oa = res.results[c]["oa"]                  # [128, 2, NB, AW] int8
        od = res.results[c]["od"]                  # [128, 2, NB, DW] int8
        bs = slice(bh * HB, (bh + 1) * HB)
        for g in range(2):
            a = oa[:, g].transpose(1, 2, 0)        # [NB, AW, 128]
            d = od[:, g].transpose(1, 2, 0)        # [NB, DW, 128]
            ok[:, 0:AW] = np.where(a >= 0, one, neg)
            ok[:, AW:HB] = np.where(d > 0, one, neg)
            j0 = jq * JS + g * 128
            full[:, bs, j0:j0 + 128] = ok
    return full.reshape(NB * B, H)
